# revision 29
# baseline (speedup 1.0000x reference)
"""Trainium2 Bass kernel for a 2-layer dense GAT (nn_GAT_70446053589175).

kernel(**inputs) takes the FULL unsharded inputs (as produced by
setup_inputs) and returns the FULL [4096, 128] float32 output.

Sharding (8 NeuronCores, single SPMD program):
  Layer 1: 2 row-groups x 4 head-groups  (2048 rows, 2 heads per core)
  Layer 2: 8-way row split (512 rows per core)

Math (layer 1, per head): att_ij = softmax_j(leakyrelu_0.2(s1_i + s2_j))
masked by adj. Using softmax shift/scale invariance, the per-row factor
exp(0.8*s1_i)... wait, more precisely:
  p_ij = exp(lrelu(s1_i + s2_j)) = max(e^{s1+s2}, e^{0.2(s1+s2)})
       = u1_i * max(w1_i * e2_j, u2_j),   w1 = e^{0.8 s1}, u = e^{0.2 s},
         e2 = e^{s2}.
The row factor u1_i cancels in the softmax ratio, so per (j-tile, head)
only TWO fast DVE ops are needed:
  g = tensor_scalar(w1rep, e2_j, u2_j, mult, max)   (4x DVE mode)
  q = g * mask                                       (2x DVE)
All [N,N]-sized elementwise work runs on the Vector engine only --
GpSimd tensor_tensor is ~4x slower AND its SBUF traffic inflates
concurrent DVE ops ~5x (measured 812ns -> 4.5us), so offloading to it
is net-negative. PSUM evacuation + small copies go to the Scalar (ACT)
engine (1/Z stays on DVE reciprocal: the custom-DVE approx variant
NaN'd on hardware when fed the PSUM Z-row). S and Wh share one fused
matmul per (jt, ft) so the xT stationary operand is loaded once.
Aggregation runs "flipped" (lhsT = q slices, rhs = [Wh | 1]) so psum
tiles are [128 rows, 65] and Z is a per-partition column -> normalize
with a single tensor_scalar divide.  Between layers, each core computes
its partial Wh2' = h_local @ [W_out | a1 | a2] (rows of its row-group)
and the cores exchange [N, 130] via ReduceScatter(4-groups) +
AllGather(8) -- 4x less collective traffic than exchanging h itself.
Layer 2 uses the same two-op attention chain.
"""
import sys
import os

for _p in ("/opt/trn_rl_repo", "/opt/pypackages",
           os.path.expanduser("~/.axon_site/_ro/trn_rl_repo"),
           os.path.expanduser("~/.axon_site/_ro/pypackages")):
    if os.path.isdir(_p) and _p not in sys.path:
        sys.path.insert(0, _p)

from contextlib import ExitStack

import numpy as np
import ml_dtypes

import concourse.bacc as bacc
import concourse.tile as tile
from concourse import mybir
from concourse.bass_utils import run_bass_kernel_spmd

dt = mybir.dt
AF = mybir.ActivationFunctionType
OP = mybir.AluOpType

BF16 = ml_dtypes.bfloat16
SLOPE = 0.2
_CACHE = {}


def _build(N=4096, F=512, D=64, H=8, O=128, n_cores=8, R=2, reps=1):
    HG = n_cores // R           # 4 head-groups
    NHL = H // HG               # 2 heads per core
    ROWS = N // R               # 2048 rows per core (layer 1)
    ROWS2 = N // n_cores        # 512 rows per core (layer 2)
    JT = N // 128               # 32 j-tiles
    FT = F // 128               # 4 feature tiles
    IT = ROWS // 128            # 16 i-tiles (layer 1)
    IT2 = ROWS2 // 128          # 4 i-tiles (layer 2)
    DL = NHL * D                # 128 local feature width
    CW = O + 2                  # 130: [Wh2 | s1 | s2] exchange width
    assert DL == 128 and O == 128

    nc = bacc.Bacc("TRN2", target_bir_lowering=False, debug=False,
                   num_devices=n_cores)

    xT_in = nc.dram_tensor("xT", [F, N], dt.bfloat16, kind="ExternalInput").ap()
    sel_in = nc.dram_tensor("sel", [JT, ROWS // 128], dt.bfloat16, kind="ExternalInput").ap()
    adjT_in = nc.dram_tensor("adjT", [N, ROWS], dt.bfloat16, kind="ExternalInput").ap()
    adjT2_in = nc.dram_tensor("adjT2", [N, ROWS2], dt.bfloat16, kind="ExternalInput").ap()
    wloc_in = nc.dram_tensor("wloc", [F, DL], dt.bfloat16, kind="ExternalInput").ap()
    wtloc_in = nc.dram_tensor("wtloc", [DL, F], dt.bfloat16, kind="ExternalInput").ap()
    ablk_in = nc.dram_tensor("ablk", [DL, 2 * NHL], dt.bfloat16, kind="ExternalInput").ap()
    wouta_in = nc.dram_tensor("wouta", [DL, CW], dt.bfloat16, kind="ExternalInput").ap()
    onesb_in = nc.dram_tensor("onesb", [1, 128], dt.bfloat16, kind="ExternalInput").ap()
    identb_in = nc.dram_tensor("identb", [128, 128], dt.bfloat16, kind="ExternalInput").ap()
    out_ext = nc.dram_tensor("out", [ROWS2, O], dt.float32, kind="ExternalOutput").ap()

    nbuf = min(reps, 2)
    partial_ds = [nc.dram_tensor(f"partial_d{b}", [ROWS, CW], dt.bfloat16)
                  for b in range(nbuf)]
    hred_ds = [nc.dram_tensor(f"hred_d{b}", [ROWS2, CW], dt.bfloat16)
               for b in range(nbuf)]
    hall_ds = [nc.dram_tensor(f"hall_d{b}", [N, CW], dt.bfloat16,
                              addr_space="Shared") for b in range(nbuf)]

    rs_groups = [list(range(g * HG, (g + 1) * HG)) for g in range(R)]

    with tile.TileContext(nc) as tc, ExitStack() as top:
        cpool = top.enter_context(tc.tile_pool(name="const", bufs=1))
        onesb_sb = cpool.tile([1, 128], dt.bfloat16, tag="onesb", name="onesb")
        nc.sync.dma_start(onesb_sb[:], onesb_in)
        identb_sb = cpool.tile([128, 128], dt.bfloat16, tag="identb", name="identb")
        nc.sync.dma_start(identb_sb[:], identb_in)
        ones2d_sb = cpool.tile([128, 64], dt.bfloat16, tag="ones2d", name="ones2d")
        nc.vector.memset(ones2d_sb[:], 1.0)

        a2pf = top.enter_context(tc.tile_pool(name="a2pf", bufs=2))
        adjT2_dram = adjT2_in.rearrange("(j p) i -> p j i", p=128)

        # ---- resident parameters + features (loaded once, reused per rep) --
        xt_pool = top.enter_context(tc.tile_pool(name="xt", bufs=1))
        w_pool = top.enter_context(tc.tile_pool(name="wp", bufs=1))
        xT_sb = [xt_pool.tile([128, N], dt.bfloat16, tag=f"xt{ft}",
                              name=f"xt{ft}") for ft in range(FT)]
        W_sb = [w_pool.tile([128, DL + 2 * NHL], dt.bfloat16, tag=f"wl{ft}",
                            name=f"wl{ft}") for ft in range(FT)]
        sel_sb = w_pool.tile([JT, ROWS // 128], dt.bfloat16, tag="sel",
                             name="sel")
        nc.scalar.dma_start(sel_sb[:], sel_in)
        wouta_sb = [w_pool.tile([D, CW], dt.bfloat16, tag=f"wouta{h}",
                                name=f"wouta{h}") for h in range(NHL)]
        for h in range(NHL):
            nc.scalar.dma_start(wouta_sb[h][:], wouta_in[D * h:D * (h + 1), :])
        for ft in range(FT):
            nc.scalar.dma_start(xT_sb[ft][:], xT_in[128 * ft:128 * (ft + 1), :])
        for ft in range(FT):
            nc.scalar.dma_start(W_sb[ft][:, 0:DL],
                                wloc_in[128 * ft:128 * (ft + 1), :])
        with ExitStack() as wt_stack:
            wtp = wt_stack.enter_context(tc.tile_pool(name="wtp", bufs=1))
            WT_sb = wtp.tile([DL, F], dt.bfloat16, tag="wtl", name="wtl")
            nc.scalar.dma_start(WT_sb[:], wtloc_in)
            A_sb = wtp.tile([DL, 2 * NHL], dt.bfloat16, tag="ablk", name="ablk")
            nc.scalar.dma_start(A_sb[:], ablk_in)
            ps_wt = wt_stack.enter_context(
                tc.tile_pool(name="pswt", bufs=1, space="PSUM"))
            wt_ps = ps_wt.tile([128, FT * 2 * NHL], dt.float32,
                               tag="wtps", name="wtps")
            for ft in range(FT):
                nc.tensor.matmul(wt_ps[:, ft * 2 * NHL:(ft + 1) * 2 * NHL],
                                 WT_sb[:, 128 * ft:128 * (ft + 1)], A_sb[:],
                                 start=True, stop=True)
            for ft in range(FT):
                nc.vector.tensor_copy(
                    W_sb[ft][:, DL:DL + 2 * NHL],
                    wt_ps[:, ft * 2 * NHL:(ft + 1) * 2 * NHL])

        def _issue_ag(hred_d, hall_d):
            nc.gpsimd.collective_compute(
                "AllGather", OP.bypass, replica_groups=[list(range(n_cores))],
                ins=[hred_d.ap()], outs=[hall_d.ap()])

        def _l2_stage(hred_d, hall_d, a2view):
            # ---- P6: own-row w12rep prep (overlaps AllGather) ----
            with ExitStack() as p6:
                l2p = p6.enter_context(tc.tile_pool(name="l2p", bufs=1))

                hred_sb = l2p.tile([128, IT2 * CW], dt.bfloat16, tag="hred",
                                   name="hred")
                nc.sync.dma_start(
                    hred_sb[:].rearrange("p (k c) -> p k c", k=IT2),
                    hred_d.ap().rearrange("(k p) c -> p k c", p=128))
                s1p_sb = l2p.tile([1, ROWS2], dt.bfloat16, tag="s1p", name="s1p")
                with ExitStack() as ps6:
                    ps_tk = ps6.enter_context(tc.tile_pool(name="pstk", bufs=2,
                                                           space="PSUM"))
                    ps_rw = ps6.enter_context(tc.tile_pool(name="psrw", bufs=1,
                                                           space="PSUM"))
                    for k in range(IT2):
                        tk_ps = ps_tk.tile([1, 128], dt.bfloat16, tag="tkps",
                                           name="tkps")
                        nc.tensor.matmul(tk_ps[:], hred_sb[:, k * CW + O:k * CW + O + 1],
                                         identb_sb[:], is_transpose=True,
                                         start=True, stop=True)
                        nc.scalar.copy(s1p_sb[:, 128 * k:128 * (k + 1)], tk_ps[:])
                    rw_ps = ps_rw.tile([128, ROWS2], dt.float32, tag="rwps",
                                       name="rwps")
                    nc.tensor.matmul(rw_ps[:], onesb_sb[:], s1p_sb[:],
                                     start=True, stop=True)
                    w12rep = l2p.tile([128, ROWS2], dt.bfloat16, tag="w12", name="w12")
                    nc.scalar.activation(w12rep[:], rw_ps[:], AF.Exp, scale=0.8)



                # ---- P7: layer-2 attention ----
                hall_sb = l2p.tile([128, JT * CW], dt.bfloat16, tag="hall",
                                   name="hall")
                hall_view = hall_sb[:].rearrange("p (j c) -> p j c", j=JT)
                hall_dram = hall_d.ap().rearrange("(j p) c -> p j c", p=128)
                e2b = l2p.tile([128, JT], dt.float32, tag="e2b", name="e2b")
                u2b = l2p.tile([128, JT], dt.float32, tag="u2b", name="u2b")
                for k in range(4):
                    sl = slice(8 * k, 8 * (k + 1))
                    nc.scalar.dma_start(hall_view[:, sl, :], hall_dram[:, sl, :])
                    nc.vector.memset(hall_view[:, sl, O:O + 1], 1.0)
                    nc.scalar.activation(
                        e2b[:, sl].rearrange("p (j one) -> p j one", one=1),
                        hall_view[:, sl, O + 1:O + 2], AF.Exp, scale=1.0)
                    nc.scalar.activation(
                        u2b[:, sl].rearrange("p (j one) -> p j one", one=1),
                        hall_view[:, sl, O + 1:O + 2], AF.Exp, scale=SLOPE)

                with ExitStack() as p7:
                    g2p = p7.enter_context(tc.tile_pool(name="g2p", bufs=4))
                    q2p = p7.enter_context(tc.tile_pool(name="q2p", bufs=6))
                    agg2 = p7.enter_context(tc.tile_pool(name="agg2", bufs=1,
                                                         space="PSUM"))
                    o2_ps = [agg2.tile([128, O + 1], dt.float32, tag=f"o2_{it}",
                                       name=f"o2_{it}") for it in range(IT2)]
                    for jt in range(JT):
                        g2 = g2p.tile([128, ROWS2], dt.bfloat16, tag="g2", name="g2")
                        nc.vector.tensor_scalar(g2[:], w12rep[:],
                                                e2b[:, jt:jt + 1], u2b[:, jt:jt + 1],
                                                op0=OP.mult, op1=OP.max)
                        q2 = q2p.tile([128, ROWS2], dt.bfloat16, tag="q2", name="q2")
                        nc.vector.tensor_tensor(q2[:], g2[:], a2view[:, jt, :], OP.mult)
                        for it in range(IT2):
                            nc.tensor.matmul(o2_ps[it][:],
                                             q2[:, 128 * it:128 * (it + 1)],
                                             hall_view[:, jt, 0:O + 1],
                                             start=(jt == 0), stop=(jt == JT - 1))
                    fo = p7.enter_context(tc.tile_pool(name="fo", bufs=4))
                    for it in range(IT2):
                        rv = fo.tile([128, 1], dt.float32, tag="rv", name="rv")
                        nc.vector.reciprocal(rv[:], o2_ps[it][:, O:O + 1])
                        ot = fo.tile([128, O], dt.float32, tag="ot", name="ot")
                        nc.vector.tensor_scalar(ot[:], o2_ps[it][:, 0:O],
                                                rv[:, 0:1], None, op0=OP.mult)
                        if reps == 1:
                            nc.sync.dma_start(out_ext[128 * it:128 * (it + 1), :], ot[:])
                        else:
                            # timing builds: accumulate so repeated bodies stay live
                            nc.gpsimd.dma_start(out_ext[128 * it:128 * (it + 1), :],
                                                ot[:], accum_op=OP.add)

        # Software pipeline: rep r's body issues AG(r-1) first (runs on the
        # CC cores during P1(r)), then P1(r), then the previous rep's L2
        # stage (all its collective inputs have landed by then), then the
        # attention body.  In-order engine queues never stall on a
        # collective in steady state.
        prev = None
        for _rep in range(reps):
            partial_d = partial_ds[_rep % nbuf]
            hred_d = hred_ds[_rep % nbuf]
            hall_d = hall_ds[_rep % nbuf]
            if prev is not None:
                _issue_ag(prev[0], prev[1])
            adjT2_sb = a2pf.tile([128, JT * ROWS2], dt.bfloat16,
                                 tag="adjt2sb", name="adjt2sb")
            a2view = adjT2_sb[:].rearrange("p (j i) -> p j i", j=JT)
            with ExitStack() as l1s:
                l1pool = l1s.enter_context(tc.tile_pool(name="l1p", bufs=1))
                Wh_sb = l1pool.tile([128, JT * NHL * (D + 1)], dt.bfloat16,
                                    tag="whsb", name="whsb")
                S_sb = l1pool.tile([128, JT * 2 * NHL], dt.bfloat16, tag="ssb", name="ssb")
                e2_sb = l1pool.tile([128, NHL * JT], dt.float32, tag="e2sb", name="e2sb")
                u2_sb = l1pool.tile([128, NHL * JT], dt.float32, tag="u2sb", name="u2sb")
                w1rep = l1pool.tile([128, NHL * ROWS], dt.bfloat16, tag="w1rep",
                                    name="w1rep")

                # ---- P1: fused Wh|S from resident xT/W tiles ----
                with ExitStack() as p1:
                    # fused Wh|S matmuls: one xT weight-load per (jt, ft)
                    wh_view = Wh_sb[:].rearrange("p (j h d) -> p j h d", j=JT, h=NHL)
                    nc.vector.memset(wh_view[:, :, :, D:D + 1], 1.0)
                    with ExitStack() as whs_stack:
                        ps_whs0 = whs_stack.enter_context(
                            tc.tile_pool(name="pswhs", bufs=3, space="PSUM"))
                        for jt in range(JT):
                            wh_ps = ps_whs0.tile([128, DL + 2 * NHL], dt.float32,
                                                 tag="whps", name="whps")
                            for ft in range(FT):
                                nc.tensor.matmul(wh_ps[:],
                                                 xT_sb[ft][:, 128 * jt:128 * (jt + 1)],
                                                 W_sb[ft][:],
                                                 start=(ft == 0), stop=(ft == FT - 1))
                            src_wh = wh_ps[:, 0:DL].rearrange("p (h d) -> p h d",
                                                              h=NHL)
                            nc.scalar.copy(wh_view[:, jt, :, 0:D], src_wh)
                            nc.vector.tensor_copy(
                                S_sb[:, jt * 2 * NHL:(jt + 1) * 2 * NHL],
                                wh_ps[:, DL:DL + 2 * NHL])
                    # e2/u2 chunked (8 j-tiles per activation) so jt=0 unblocks early
                    s_view = S_sb[:].rearrange("p (j c) -> p j c", j=JT)
                    e2_view = e2_sb[:].rearrange("p (h j) -> p h j", h=NHL)
                    u2_view = u2_sb[:].rearrange("p (h j) -> p h j", h=NHL)
                    for ck in range(4):
                        js = slice(8 * ck, 8 * (ck + 1))
                        for h in range(NHL):
                            nc.scalar.activation(
                                e2_view[:, h:h + 1, js].rearrange("p one j -> p j one"),
                                s_view[:, js, 2 * h + 1:2 * h + 2], AF.Exp, scale=1.0)
                            nc.scalar.activation(
                                u2_view[:, h:h + 1, js].rearrange("p one j -> p j one"),
                                s_view[:, js, 2 * h + 1:2 * h + 2], AF.Exp, scale=SLOPE)

                    # own-row s1: transpose S, row-select via sel columns, then
                    # replicate + exp(0.8 s1) -> w1rep. No SBUF-to-SBUF DMAs.
                    with ExitStack() as st_stack:
                        ps_tr1 = st_stack.enter_context(
                            tc.tile_pool(name="pstr1", bufs=2, space="PSUM"))
                        ps_row = st_stack.enter_context(
                            tc.tile_pool(name="psrow", bufs=1, space="PSUM"))
                        ps_rep = st_stack.enter_context(
                            tc.tile_pool(name="psrep", bufs=2, space="PSUM"))
                        for h in range(NHL):
                            smt_ps = ps_tr1.tile([JT, 128], dt.bfloat16, tag="smtps",
                                                 name="smtps")
                            nc.tensor.matmul(
                                smt_ps[:],
                                s_view[:, :, 2 * h:2 * h + 1].rearrange(
                                    "p j one -> p (j one)"),
                                identb_sb[:], is_transpose=True, start=True, stop=True)
                            smt_sb = l1pool.tile([JT, 128], dt.bfloat16, tag=f"smt{h}",
                                                 name=f"smt{h}")
                            nc.scalar.copy(smt_sb[:], smt_ps[:])
                            row_ps = ps_row.tile([1, ROWS], dt.float32, tag="rowps",
                                                 name="rowps")
                            for k in range(ROWS // 128):
                                nc.tensor.matmul(row_ps[:, 128 * k:128 * (k + 1)],
                                                 sel_sb[:, k:k + 1], smt_sb[:],
                                                 start=True, stop=True)
                            s1row = l1pool.tile([1, ROWS], dt.bfloat16, tag=f"s1r{h}",
                                                name=f"s1r{h}")
                            for icx in range(ROWS // 512):
                                o = 512 * icx
                                if icx % 2:
                                    nc.scalar.copy(s1row[:, o:o + 512],
                                                   row_ps[:, o:o + 512])
                                else:
                                    nc.vector.tensor_copy(s1row[:, o:o + 512],
                                                          row_ps[:, o:o + 512])
                            for icx in range(ROWS // 512):
                                rep_ps = ps_rep.tile([128, 512], dt.float32,
                                                     tag="repps", name="repps")
                                nc.tensor.matmul(rep_ps[:], onesb_sb[:],
                                                 s1row[:, 512 * icx:512 * (icx + 1)],
                                                 start=True, stop=True)
                                nc.scalar.activation(
                                    w1rep[:, h * ROWS + 512 * icx:h * ROWS + 512 * (icx + 1)],
                                    rep_ps[:], AF.Exp, scale=0.8)


                if prev is not None:
                    _l2_stage(prev[0], prev[1], prev[2])

                # ---- P3: layer-1 attention + aggregation ([65, 512] psum) ----
                p45 = l1s.enter_context(tc.tile_pool(name="p45", bufs=1))
                IC = ROWS // 512
                full_sb = p45.tile([D + 1, NHL * ROWS], dt.bfloat16, tag="fullsb",
                                   name="fullsb")
                with ExitStack() as p3:
                    mpool = p3.enter_context(tc.tile_pool(name="mt", bufs=5))
                    gpool = p3.enter_context(tc.tile_pool(name="gpl", bufs=3))
                    qpool = p3.enter_context(tc.tile_pool(name="qpl", bufs=4))
                    agg = p3.enter_context(tc.tile_pool(name="agg", bufs=1, space="PSUM"))
                    agg_ps = [[agg.tile([D + 1, 512], dt.float32, tag=f"agg{h}_{icx}",
                                        name=f"agg{h}_{icx}") for icx in range(IC)]
                              for h in range(NHL)]

                    for jt in range(JT):
                        mt = mpool.tile([128, ROWS], dt.bfloat16, tag="mt", name="mt")
                        nc.sync.dma_start(mt[:], adjT_in[128 * jt:128 * (jt + 1), :])
                        # prefetch adjT2 in 4 chunks mid-loop (overlaps compute)
                        if jt in (12, 17, 22, 27):
                            k = (jt - 12) // 5
                            nc.sync.dma_start(a2view[:, 8 * k:8 * (k + 1), :],
                                              adjT2_dram[:, 8 * k:8 * (k + 1), :])
                        for h in range(NHL):
                            g_t = gpool.tile([128, ROWS], dt.bfloat16, tag="g", name="g")
                            nc.vector.tensor_scalar(
                                g_t[:], w1rep[:, h * ROWS:(h + 1) * ROWS],
                                e2_sb[:, h * JT + jt:h * JT + jt + 1],
                                u2_sb[:, h * JT + jt:h * JT + jt + 1],
                                op0=OP.mult, op1=OP.max)
                            q_t = qpool.tile([128, ROWS], dt.bfloat16, tag="q", name="q")
                            nc.vector.tensor_tensor(q_t[:], g_t[:], mt[:], OP.mult)
                            for icx in range(IC):
                                nc.tensor.matmul(
                                    agg_ps[h][icx][:],
                                    wh_view[:, jt, h, :],
                                    q_t[:, 512 * icx:512 * (icx + 1)],
                                    start=(jt == 0), stop=(jt == JT - 1))

                    # evacuate [raw | Z] to SBUF on ACT (keep DVE free),
                    # then 1/Z in place on the Z row (partition D)
                    with nc.allow_low_precision(reason="1/Z in bf16; tol 2e-2"):
                        for h in range(NHL):
                            for icx in range(IC):
                                o = h * ROWS + 512 * icx
                                nc.scalar.copy(full_sb[:, o:o + 512],
                                               agg_ps[h][icx][:])
                        # 1/Z on ACT as exp(-ln(Z)) -- keeps DVE free
                        # (~1e-4 rel err from the act tables; tol is 2e-2).
                        # Two wide [1, 4096] ops: the 224-cycle ACT overhead
                        # made 32 narrow [1, 512] ops cost ~18us vs ~4us.
                        nc.scalar.activation(full_sb[D:D + 1, :],
                                             full_sb[D:D + 1, :], AF.Ln)
                        nc.scalar.activation(full_sb[D:D + 1, :],
                                             full_sb[D:D + 1, :],
                                             AF.Exp, scale=-1.0)

                # ---- P4: h = elu(raw / Z) ----
                with ExitStack() as p5:
                    ps_zr = p5.enter_context(tc.tile_pool(name="pszr", bufs=3,
                                                          space="PSUM"))
                    t_sb = p45.tile([D, NHL * ROWS], dt.bfloat16, tag="tsb", name="tsb")
                    for h in range(NHL):
                        for icx in range(IC):
                            o = h * ROWS + 512 * icx
                            zr_ps = ps_zr.tile([D, 512], dt.float32, tag="zrps",
                                               name="zrps")
                            nc.tensor.matmul(zr_ps[:], ones2d_sb[64:65, 0:D],
                                             full_sb[D:D + 1, o:o + 512],
                                             start=True, stop=True)
                            nc.vector.tensor_tensor(t_sb[:, o:o + 512],
                                                    full_sb[0:D, o:o + 512],
                                                    zr_ps[:], OP.mult)
                    # elu(t) = exp(min(t,0)) + (relu(t) - 1); exp in place
                    # on m0, result in place on t_sb (both fully consumed)
                    m0 = p45.tile([D, NHL * ROWS], dt.bfloat16, tag="m0", name="m0")
                    d1 = p45.tile([D, NHL * ROWS], dt.bfloat16, tag="d1", name="d1")
                    elu_sb = t_sb
                    for h in range(NHL):
                        hs = slice(h * ROWS, (h + 1) * ROWS)
                        nc.vector.tensor_scalar(m0[:, hs], t_sb[:, hs], 0.0, None,
                                                op0=OP.min)
                        nc.scalar.activation(m0[:, hs], m0[:, hs], AF.Exp)
                        nc.vector.tensor_scalar(d1[:, hs], t_sb[:, hs], 0.0, 1.0,
                                                op0=OP.max, op1=OP.subtract)
                        nc.vector.tensor_tensor(t_sb[:, hs], m0[:, hs], d1[:, hs],
                                                OP.add)

                    # ---- P5: partial Wh2' = h @ [Wout|a1|a2] (no transposes) ----
                    pout_sb = p45.tile([128, IT * CW], dt.bfloat16, tag="pout",
                                       name="pout")
                    ps_pw = p5.enter_context(tc.tile_pool(name="pspw", bufs=3,
                                                          space="PSUM"))
                    pout_dram = partial_d.ap().rearrange("(i p) c -> p i c", p=128)
                    pout_view = pout_sb[:].rearrange("p (i c) -> p i c", i=IT)
                    for it in range(IT):
                        pw_ps = ps_pw.tile([128, CW], dt.float32, tag="pwps",
                                           name="pwps")
                        for h in range(NHL):
                            nc.tensor.matmul(
                                pw_ps[:],
                                elu_sb[:, h * ROWS + 128 * it:h * ROWS + 128 * (it + 1)],
                                wouta_sb[h], start=(h == 0), stop=(h == NHL - 1))
                        nc.scalar.copy(pout_sb[:, CW * it:CW * (it + 1)], pw_ps[:])
                        if it == IT // 2 - 1:
                            nc.sync.dma_start(pout_dram[:, 0:IT // 2, :],
                                              pout_view[:, 0:IT // 2, :])
                    nc.sync.dma_start(pout_dram[:, IT // 2:, :],
                                      pout_view[:, IT // 2:, :])

            # ---- P5b: ReduceScatter within row-groups ----
            nc.gpsimd.collective_compute(
                "ReduceScatter", OP.add, replica_groups=rs_groups,
                ins=[partial_d.ap()], outs=[hred_d.ap()])
            prev = (hred_d, hall_d, a2view)

        _issue_ag(prev[0], prev[1])
        _l2_stage(prev[0], prev[1], prev[2])

    nc.compile()
    return nc




def _make_in_maps(x, adj, W_heads, a1_heads, a2_heads, W_out, a1_out, a2_out,
                  n_cores=8, R=2):
    N, F = x.shape
    H, _, D = W_heads.shape
    O = W_out.shape[1]
    HG = n_cores // R
    NHL = H // HG
    ROWS = N // R
    ROWS2 = N // n_cores

    xT = np.ascontiguousarray(x.T).astype(BF16)
    adjT = np.ascontiguousarray(adj.T).astype(BF16)
    onesb = np.ones((1, 128), np.float32).astype(BF16)
    identb = np.eye(128, dtype=np.float32).astype(BF16)

    in_maps = []
    for c in range(n_cores):
        rr, hg = c // HG, c % HG
        heads = list(range(NHL * hg, NHL * (hg + 1)))
        r0 = ROWS * rr
        wloc = np.concatenate([W_heads[h] for h in heads], axis=1).astype(BF16)
        wtloc = np.concatenate([W_heads[h].T for h in heads], axis=0).astype(BF16)
        ablk = np.zeros((NHL * D, 2 * NHL), np.float32)
        for k, h in enumerate(heads):
            ablk[k * D:(k + 1) * D, 2 * k] = a1_heads[h]
            ablk[k * D:(k + 1) * D, 2 * k + 1] = a2_heads[h]
        d0 = NHL * D * hg
        wa1 = (W_out @ a1_out)[d0:d0 + NHL * D]
        wa2 = (W_out @ a2_out)[d0:d0 + NHL * D]
        wouta = np.concatenate(
            [W_out[d0:d0 + NHL * D, :], wa1[:, None], wa2[:, None]],
            axis=1).astype(BF16)
        sel = np.zeros((N // 128, ROWS // 128), np.float32)
        for k in range(ROWS // 128):
            sel[rr * (ROWS // 128) + k, k] = 1.0
        in_maps.append({
            "xT": xT,
            "sel": sel.astype(BF16),
            "adjT": np.ascontiguousarray(adjT[:, r0:r0 + ROWS]),
            "adjT2": np.ascontiguousarray(adjT[:, ROWS2 * c:ROWS2 * (c + 1)]),
            "wloc": wloc,
            "wtloc": wtloc,
            "ablk": ablk.astype(BF16),
            "wouta": wouta,
            "onesb": onesb,
            "identb": identb,
        })
    return in_maps


def kernel(x, adj, W_heads, a1_heads, a2_heads, W_out, a1_out, a2_out):
    x = np.asarray(x, dtype=np.float32)
    adj = np.asarray(adj)
    W_heads = np.asarray(W_heads, dtype=np.float32)
    a1_heads = np.asarray(a1_heads, dtype=np.float32)
    a2_heads = np.asarray(a2_heads, dtype=np.float32)
    W_out = np.asarray(W_out, dtype=np.float32)
    a1_out = np.asarray(a1_out, dtype=np.float32)
    a2_out = np.asarray(a2_out, dtype=np.float32)

    if "nc" not in _CACHE:
        _CACHE["nc"] = _build()
    nc = _CACHE["nc"]
    in_maps = _make_in_maps(x, adj, W_heads, a1_heads, a2_heads,
                            W_out, a1_out, a2_out)
    res = run_bass_kernel_spmd(nc, in_maps, list(range(8)))
    out = np.concatenate([r["out"] for r in res.results], axis=0)
    return out.astype(np.float32)


if __name__ == "__main__":
    import jax
    key = jax.random.key(0)
    ks = jax.random.split(key, 8)
    import jax.numpy as jnp
    N, F, D, H, O = 4096, 512, 64, 8, 128
    ins = {
        "x": np.asarray(jax.random.normal(ks[0], (N, F), dtype=jnp.float32)),
        "adj": np.asarray(jax.random.randint(ks[1], (N, N), 0, 2, dtype=jnp.int32)),
        "W_heads": np.asarray(jax.random.normal(ks[2], (H, F, D), dtype=jnp.float32) * 0.05),
        "a1_heads": np.asarray(jax.random.normal(ks[3], (H, D), dtype=jnp.float32) * 0.05),
        "a2_heads": np.asarray(jax.random.normal(ks[4], (H, D), dtype=jnp.float32) * 0.05),
        "W_out": np.asarray(jax.random.normal(ks[5], (H * D, O), dtype=jnp.float32) * 0.05),
        "a1_out": np.asarray(jax.random.normal(ks[6], (O,), dtype=jnp.float32) * 0.05),
        "a2_out": np.asarray(jax.random.normal(ks[7], (O,), dtype=jnp.float32) * 0.05),
    }
    out = kernel(**ins)
    print("out", out.shape, out.dtype, float(np.abs(out).max()))

